# revision 22
# baseline (speedup 1.0000x reference)
"""GraphSAGE (2-layer, mean-agg) edge-scoring kernel for 8 trn2 NeuronCores.

Per-core private compacted tables (no pair sharing, no barrier):
  - h0/h1 hop tables stored in token order (plain / transposing DMA loads).
  - hop-2 rows deduped into 2 tables per side (segs {0,1} and {2,3,4}),
    each guaranteed <= 30720 rows so plain positive int16 dma_gather idx work
    with no chunking and no reorder (gathers are order-preserving,
    1024-idx calls, k-major so the 10-way mean is 9 strided vector adds).
  - Projection reads host-prepacked fp16 feature tiles (HWDGE), writes each
    table slot contiguously; per-table dram tensors let Tile overlap SAGE
    gathers with the tail of projection.
  - SAGE matmuls run feat-major; token-major gathered data is transposed
    via TensorE (identity matmul) in packed PSUM groups.
"""
import os
import numpy as np

F0 = F1 = 10
B = 4096
NCORES = 8
EDGES = B // NCORES          # 512
P = 128
D = 256
NU, NI = 50000, 100000
SEG = 1024                   # hop-2 groups per segment
NSEG = (EDGES * F0) // SEG   # 5
NSEG_A = 2                   # segs 0,1 -> table A; segs 2.. -> table B
CALL = 1024                  # dma_gather idx per call (HW cap)
PROJ_TILE = 512


def _wrap16(a):
    a = np.asarray(a, np.int16)
    w = a.reshape(-1, 16).T
    return np.tile(w, (8, 1)).astype(np.int16)


def _pad512(n):
    return max(512, (int(n) + 511) & ~511)


def _build_plans(inputs):
    """Per-core compacted tables + k-major hop-2 gather indices.

    Returns dict with:
      sizes: dict slot -> padded row count (max over cores)
      rows:  per core: dict slot -> original-table row ids (np.int64 array)
      idx2:  per core: dict side -> [NSEG*F1*CALL] int16 gather idx (k-major)
    """
    h = {}
    for k in ("src_h0", "src_h1", "src_h2", "dst_h0", "dst_h1", "dst_h2"):
        h[k] = np.asarray(inputs[k]).astype(np.int64).reshape(NCORES, -1)

    rows = [dict() for _ in range(NCORES)]
    idx2 = [dict() for _ in range(NCORES)]
    for c in range(NCORES):
        for sd in ("s", "d"):
            pre = "src" if sd == "s" else "dst"
            h0 = h[pre + "_h0"][c]
            h1 = h[pre + "_h1"][c]
            h2 = h[pre + "_h2"][c]
            rows[c][sd + "h0"] = h0
            rows[c][sd + "h1"] = h1
            invs = {}
            for gi, segs in enumerate(_SEG_GROUPS):
                toks = np.concatenate([
                    h2[s * SEG * F1:(s + 1) * SEG * F1] for s in segs])
                u, inv = np.unique(toks, return_inverse=True)
                assert len(u) <= 32000
                rows[c][sd + "ABC"[gi]] = u
                for i, s in enumerate(segs):
                    invs[s] = inv[i * SEG * F1:(i + 1) * SEG * F1]
            calls = []
            for s in range(NSEG):
                inv = invs[s]
                for k in range(F1):
                    # call position q (=token within seg) -> table pos
                    calls.append(inv[np.arange(SEG) * F1 + k])
            idx2[c][sd] = np.concatenate(calls)
    sizes = {}
    for slot in set(_SLOTS):
        sizes[slot] = _pad512(max(len(rows[c][slot]) for c in range(NCORES)))
    return {"sizes": sizes, "rows": rows, "idx2": idx2}


# slot -> (feature kind, bias column) ; src side: h0/h2=user, h1=item
_SLOT_KIND = {"sh0": "u", "sh1": "i", "sA": "u", "sB": "u", "sC": "u",
              "dh0": "i", "dh1": "u", "dA": "i", "dB": "i", "dC": "i"}
# hop-2 seg groups -> tables A/B/C (finer projection->gather gating)
_SEG_GROUPS = ((0, 1), (2, 3), (4,))
_SEG_TO_TAB = {0: "A", 1: "A", 2: "B", 3: "B", 4: "C"}
# all hop-2 tables first so every gather overlaps remaining projection
_SLOTS = ("sA", "dA", "sB", "dB", "sC", "dC", "sh1", "dh1", "sh0", "dh0")


def _proj_host(feat_u16, feat_i16, plan):
    """Build per-core xt: [512, TOT] fp16, feature-major, tile-permuted.

    Within each 512-col tile, col q = cdim*128 + m holds row m*4 + cdim of
    the slot (so the projected PSUM groups store as contiguous row spans).
    """
    sizes = plan["sizes"]
    xts = []
    for c in range(NCORES):
        parts = []
        for slot in _SLOTS:
            r = plan["rows"][c][slot]
            n = sizes[slot]
            cols = np.zeros((n,), np.int64)
            cols[:len(r)] = r
            perm = cols.reshape(n // PROJ_TILE, P, 4).transpose(
                0, 2, 1).reshape(-1)
            feat = feat_u16 if _SLOT_KIND[slot] == "u" else feat_i16
            parts.append(feat[:, perm])
        xts.append(np.ascontiguousarray(np.concatenate(parts, axis=1)))
    return xts


def _build_bass(plan, debug=False):
    import concourse.tile as tile
    import concourse.bacc as bacc
    from concourse import mybir, library_config
    from concourse.masks import make_identity
    from contextlib import ExitStack

    f16 = mybir.dt.float16
    f32 = mybir.dt.float32
    i16 = mybir.dt.int16
    AF = mybir.ActivationFunctionType

    sizes = plan["sizes"]
    tot = sum(sizes[s] for s in _SLOTS)

    nc = bacc.Bacc("TRN2", target_bir_lowering=False, debug=False,
                   num_devices=NCORES, num_swdge_queues=4)

    xt = nc.dram_tensor("xt", [512, tot], f16, kind="ExternalInput")
    w_pu = nc.dram_tensor("w_pu", [P, 4, D], f16, kind="ExternalInput")
    w_pi = nc.dram_tensor("w_pi", [P, 4, D], f16, kind="ExternalInput")
    b_p = nc.dram_tensor("b_p", [1, 2, 2 * D], f16, kind="ExternalInput")
    wsage = nc.dram_tensor("wsage", [P, 2, 2 * 768], f16, kind="ExternalInput")
    wlin = nc.dram_tensor("wlin", [P, 1], f16, kind="ExternalInput")
    blin = nc.dram_tensor("blin", [1, 1], f32, kind="ExternalInput")
    idx_s = nc.dram_tensor("idx_s", [P, NSEG * F1 * CALL // 16], i16,
                           kind="ExternalInput")
    idx_d = nc.dram_tensor("idx_d", [P, NSEG * F1 * CALL // 16], i16,
                           kind="ExternalInput")
    out = nc.dram_tensor("out", [1, EDGES], f32, kind="ExternalOutput")

    tabs = {s: nc.dram_tensor(f"tab_{s}", [sizes[s], D], f16)
            for s in _SLOTS}

    with tile.TileContext(nc) as tc, ExitStack() as ctx:
        nc.gpsimd.load_library(library_config.mlp)
        tc.strict_bb_all_engine_barrier()

        wpool = ctx.enter_context(tc.tile_pool(name="w", bufs=1))
        w_pu_s = wpool.tile([P, 4, D], f16, tag="wpu")
        w_pi_s = wpool.tile([P, 4, D], f16, tag="wpi")
        b_p_s = wpool.tile([1, 2, 2 * D], f16, tag="bp")
        wsage_s = wpool.tile([P, 2, 2 * 768], f16, tag="wsage")
        wlin_s = wpool.tile([P, 1], f16, tag="wlin")
        blin_s = wpool.tile([1, 1], f32, tag="blin")
        ones_s = wpool.tile([1, P], f16, tag="ones")
        ident = wpool.tile([P, P], f16, tag="ident")
        for dst_, src_ in ((w_pu_s, w_pu), (w_pi_s, w_pi), (b_p_s, b_p),
                           (wsage_s, wsage), (wlin_s, wlin), (blin_s, blin)):
            nc.sync.dma_start(dst_[:], src_[:])
        nc.vector.memset(ones_s[:], 1.0)
        make_identity(nc, ident[:])

        # ---------------- phase A: projection ----------------
        ppool = ctx.enter_context(tc.tile_pool(name="proj", bufs=2))
        pspool = ctx.enter_context(tc.tile_pool(name="pps", bufs=2,
                                                space="PSUM"))
        col_off = 0
        for slot in _SLOTS:
            tab = tabs[slot]
            kind = _SLOT_KIND[slot]
            w_s = w_pu_s if kind == "u" else w_pi_s
            bcol = 0 if kind == "u" else 1
            ntiles = sizes[slot] // PROJ_TILE
            for t in range(ntiles):
                base = col_off + t * PROJ_TILE
                xtt = ppool.tile([P, 4, PROJ_TILE], f16, tag="xtt")
                nc.sync.dma_start(
                    xtt[:],
                    xt[:, base:base + PROJ_TILE].rearrange(
                        "(c p) n -> p c n", p=P))
                sig = ppool.tile([P, 4, D], f16, tag="sig")
                sgl = sig[:].rearrange("p a b -> p (a b)")
                # one PSUM bank per 2 j-groups; bias matmul opens the
                # accumulation group over the whole bank, mains accumulate
                for hb in range(2):
                    ps = pspool.tile([P, 2, D], f32, tag="pps")
                    psl = ps[:].rearrange("p a b -> p (a b)")
                    nc.tensor.matmul(out=psl[:, :], lhsT=ones_s[:, :],
                                     rhs=b_p_s[:, bcol, :], start=True,
                                     stop=False)
                    for jj in range(2):
                        j = 2 * hb + jj
                        for cch in range(4):
                            nc.tensor.matmul(
                                out=ps[:, jj, :],
                                lhsT=xtt[:, cch, j * P:(j + 1) * P],
                                rhs=w_s[:, cch, :], start=False,
                                stop=(cch == 3))
                    nc.scalar.activation(out=sgl[:, hb * 512:(hb + 1) * 512],
                                         in_=psl[:, :], func=AF.Sigmoid)
                nc.scalar.dma_start(
                    tab[t * PROJ_TILE:(t + 1) * PROJ_TILE, :].rearrange(
                        "(p r) d -> p (r d)", r=4),
                    sig[:].rearrange("p r d -> p (r d)"))
            col_off += sizes[slot]

        # ---------------- phase C: SAGE ----------------
        hpool = ctx.enter_context(tc.tile_pool(name="hts", bufs=1))
        # shared scratch: [P,8,256] f32 is the largest shape under this tag
        vpool = ctx.enter_context(tc.tile_pool(name="vtmp", bufs=3))
        plpool = ctx.enter_context(tc.tile_pool(name="pl", bufs=4))
        acpool = ctx.enter_context(tc.tile_pool(name="ac", bufs=2))
        i2pool = ctx.enter_context(tc.tile_pool(name="i2p", bufs=2))
        gpool = ctx.enter_context(tc.tile_pool(name="gts", bufs=1))
        ps2 = ctx.enter_context(tc.tile_pool(name="ps2", bufs=2, space="PSUM"))
        pst = ctx.enter_context(tc.tile_pool(name="pst", bufs=2, space="PSUM"))

        qn = [0]

        def tree10(src, dst, ngr):
            # src [P,2,ngr*10] f16 (col g*10+k) -> dst [P,2,ngr] f32 sums
            t0_t = vpool.tile([P, 2, EDGES], f32, tag="tr")
            t0 = t0_t[:, :, :ngr]
            t1_t = vpool.tile([P, 2, EDGES], f32, tag="tr")
            t1 = t1_t[:, :, :ngr]
            v = src.rearrange("p c (j k) -> p c j k", k=F0)
            nc.vector.tensor_add(t0[:], v[:, :, :, 0], v[:, :, :, 1])
            for i in range(1, 5):
                nc.vector.tensor_add(t1[:], v[:, :, :, 2 * i],
                                     v[:, :, :, 2 * i + 1])
                if i < 4:
                    nc.vector.tensor_add(t0[:], t0[:], t1[:])
            nc.vector.tensor_add(dst, t0[:], t1[:])

        hts = {}
        for si, sd in enumerate(("s", "d")):
            wof = si * 768
            ws0 = wsage_s[:, :, wof:wof + D]
            wa0 = wsage_s[:, :, wof + D:wof + 2 * D]
            ws1 = wsage_s[:, :, wof + 2 * D:wof + 2 * D + 128]
            wa1 = wsage_s[:, :, wof + 2 * D + 128:wof + 768]
            tab_h0 = tabs[sd + "h0"]
            tab_h1 = tabs[sd + "h1"]

            # --- h0T / h1T via transposing DMA loads (token order tables) ---
            h0T = hpool.tile([P, 2, EDGES], f16, tag="h0T")
            h1T = hpool.tile([P, 2, EDGES * F0], f16, tag="h1T")
            for f in range(2):
                nc.sync.dma_start_transpose(
                    h0T[:, f, :], tab_h0[:, f * P:(f + 1) * P])
                nc.sync.dma_start_transpose(
                    h1T[:, f, :], tab_h1[:, f * P:(f + 1) * P])

            # --- n0T = group-sums of h1T ---
            n0f = vpool.tile([P, 2, EDGES], f32, tag="tr")
            tree10(h1T[:], n0f[:], EDGES)
            n0T = hpool.tile([P, 2, EDGES], f16, tag="n0T")
            nc.vector.tensor_copy(n0T[:], n0f[:])

            # --- hop-2 segments -> n1T (rotating k-planes, chained accum) ---
            i2t = i2pool.tile([P, NSEG * F1 * CALL // 16], i16, tag="i2t")
            nc.sync.dma_start(i2t[:], (idx_s if sd == "s" else idx_d)[:])
            n1T = hpool.tile([P, 2, EDGES * F0], f16, tag="n1T")
            for s in range(NSEG):
                tabAB = tabs[sd + _SEG_TO_TAB[s]]
                acc = acpool.tile([P, SEG // P, D], f16, tag="acc")
                planes = []
                for k in range(F1):
                    pl = plpool.tile([P, SEG // P, D], f16, tag="pl")
                    co = (s * F1 + k) * (CALL // 16)
                    nc.gpsimd.dma_gather(
                        pl[:], tabAB[:, :], i2t[:, co:co + CALL // 16],
                        CALL, CALL, D, queue_num=qn[0] % 4)
                    qn[0] += 1
                    planes.append(pl)
                    if k == 1:
                        nc.vector.tensor_add(acc[:], planes[0][:], planes[1][:])
                    elif k > 1:
                        nc.vector.tensor_add(acc[:], acc[:], pl[:])
                n1s = acc
                # transpose to feat-major: 16 tiles packed 4-per-psum
                for f in range(2):
                    for g in range(2):
                        pt = pst.tile([P, 4, P], f16, tag="pt")
                        for b in range(4):
                            nc.tensor.matmul(
                                out=pt[:, b, :],
                                lhsT=n1s[:, g * 4 + b, f * P:(f + 1) * P],
                                rhs=ident[:], is_transpose=True)
                        dst = n1T[:, f, s * SEG + g * 512:
                                  s * SEG + (g + 1) * 512]
                        if (f + g) % 2 == 0:
                            nc.scalar.activation(out=dst, in_=pt[:].rearrange(
                                "p b q -> p (b q)"), func=AF.Copy)
                        else:
                            nc.vector.tensor_copy(dst, pt[:].rearrange(
                                "p b q -> p (b q)"))

            # --- g1 = relu(h1 @ Ws0 + n1 @ Wa0) ---
            g1T = gpool.tile([P, 2, EDGES * F0], f16, tag="g1T")
            for o in range(2):
                for bb in range(EDGES * F0 // 512):
                    sl = slice(bb * 512, (bb + 1) * 512)
                    ps = ps2.tile([P, 512], f32, tag="mmps")
                    for cch in range(2):
                        nc.tensor.matmul(
                            out=ps[:], lhsT=ws0[:, cch, o * P:(o + 1) * P],
                            rhs=h1T[:, cch, sl], start=(cch == 0), stop=False)
                        nc.tensor.matmul(
                            out=ps[:], lhsT=wa0[:, cch, o * P:(o + 1) * P],
                            rhs=n1T[:, cch, sl], start=False, stop=(cch == 1))
                    nc.scalar.activation(out=g1T[:, o, sl], in_=ps[:],
                                         func=AF.Relu)

            # --- nT = group-sums of g1T ---
            nf = vpool.tile([P, 2, EDGES], f32, tag="tr")
            tree10(g1T[:], nf[:], EDGES)
            nT = hpool.tile([P, 2, EDGES], f16, tag="nT")
            nc.vector.tensor_copy(nT[:], nf[:])

            # --- g0 = relu(h0 @ Ws0 + n0 @ Wa0) ---
            g0T = gpool.tile([P, 2, EDGES], f16, tag="g0T")
            for o in range(2):
                ps = ps2.tile([P, 512], f32, tag="mmps")
                for cch in range(2):
                    nc.tensor.matmul(out=ps[:],
                                     lhsT=ws0[:, cch, o * P:(o + 1) * P],
                                     rhs=h0T[:, cch, :], start=(cch == 0),
                                     stop=False)
                    nc.tensor.matmul(out=ps[:],
                                     lhsT=wa0[:, cch, o * P:(o + 1) * P],
                                     rhs=n0T[:, cch, :], start=False,
                                     stop=(cch == 1))
                nc.scalar.activation(out=g0T[:, o, :], in_=ps[:], func=AF.Relu)

            # --- hT = g0 @ Ws1 + n @ Wa1 ---
            ps = ps2.tile([P, 512], f32, tag="mmps")
            for cch in range(2):
                nc.tensor.matmul(out=ps[:], lhsT=ws1[:, cch, :],
                                 rhs=g0T[:, cch, :], start=(cch == 0),
                                 stop=False)
                nc.tensor.matmul(out=ps[:], lhsT=wa1[:, cch, :],
                                 rhs=nT[:, cch, :], start=False,
                                 stop=(cch == 1))
            hT = gpool.tile([P, EDGES], f16, tag=f"hT{sd}")
            nc.scalar.activation(out=hT[:], in_=ps[:], func=AF.Copy)
            hts[sd] = hT

        prod = gpool.tile([P, EDGES], f16, tag="prod")
        nc.vector.tensor_mul(prod[:], hts["s"][:], hts["d"][:])
        psf = ps2.tile([1, EDGES], f32, tag="mmps")
        nc.tensor.matmul(out=psf[:], lhsT=wlin_s[:], rhs=prod[:],
                         start=True, stop=True)
        res = gpool.tile([1, EDGES], f32, tag="res")
        nc.scalar.activation(out=res[:], in_=psf[:], func=AF.Identity,
                             bias=blin_s[:, :1])
        nc.sync.dma_start(out[:], res[:])

    nc.compile()
    return nc


def kernel(**inputs) -> np.ndarray:
    from concourse.bass_utils import run_bass_kernel_spmd

    plan = _build_plans(inputs)

    trace = bool(os.environ.get("GNN_TRACE"))
    if trace:
        import timing_shim
        timing_shim.install()

    nc = _build_bass(plan)

    f16 = np.float16
    # feature tables, feature-major fp16 [512, N]
    feat_u16 = np.ascontiguousarray(
        np.asarray(inputs["user_feat"], np.float32).T).astype(f16)
    feat_i16 = np.ascontiguousarray(
        np.asarray(inputs["item_feat"], np.float32).T).astype(f16)
    xts = _proj_host(feat_u16, feat_i16, plan)

    w_pu = np.ascontiguousarray(
        np.asarray(inputs["W_pu"], np.float32).reshape(4, P, D)
        .transpose(1, 0, 2)).astype(f16)
    w_pi = np.ascontiguousarray(
        np.asarray(inputs["W_pi"], np.float32).reshape(4, P, D)
        .transpose(1, 0, 2)).astype(f16)
    b_p = np.stack([np.tile(np.asarray(inputs["b_pu"], np.float32), 2),
                    np.tile(np.asarray(inputs["b_pi"], np.float32), 2)])[None].astype(f16)

    def sagew(pre):
        s0 = np.asarray(inputs[f"{pre}_self0"], np.float32)
        a0 = np.asarray(inputs[f"{pre}_agg0"], np.float32) * (1.0 / F0)
        s1 = np.asarray(inputs[f"{pre}_self1"], np.float32)
        a1 = np.asarray(inputs[f"{pre}_agg1"], np.float32) * (1.0 / F0)
        cat = np.concatenate([s0, a0, s1, a1], axis=1)  # [256, 768]
        return cat.reshape(2, P, 768).transpose(1, 0, 2)

    wsage = np.ascontiguousarray(
        np.concatenate([sagew("u"), sagew("i")], axis=2)).astype(f16)
    wlin = np.asarray(inputs["W_lin"], np.float32).astype(f16)
    blin = np.asarray(inputs["b_lin"], np.float32).reshape(1, 1)

    in_maps = []
    for c in range(NCORES):
        m = {
            "xt": xts[c], "w_pu": w_pu, "w_pi": w_pi, "b_p": b_p,
            "wsage": wsage, "wlin": wlin, "blin": blin,
            "idx_s": _wrap16(plan["idx2"][c]["s"]),
            "idx_d": _wrap16(plan["idx2"][c]["d"]),
        }
        in_maps.append(m)

    kw = dict(trace=True, trace_cores=list(range(NCORES))) if trace else {}
    res = run_bass_kernel_spmd(nc, in_maps, core_ids=list(range(NCORES)), **kw)
    if trace and res.exec_time_ns:
        print(f"HW exec time: {res.exec_time_ns} ns")
        kernel.last_exec_ns = res.exec_time_ns

    logits = np.concatenate([res.results[c]["out"][0] for c in range(NCORES)])
    return logits.reshape(B, 1).astype(np.float32)


# revision 23
# speedup vs baseline: 1.0191x; 1.0191x over previous
"""GraphSAGE (2-layer, mean-agg) edge-scoring kernel for 8 trn2 NeuronCores.

Per-core private compacted tables (no pair sharing, no barrier):
  - h0/h1 hop tables stored in token order (plain / transposing DMA loads).
  - hop-2 rows deduped into 2 tables per side (segs {0,1} and {2,3,4}),
    each guaranteed <= 30720 rows so plain positive int16 dma_gather idx work
    with no chunking and no reorder (gathers are order-preserving,
    1024-idx calls, k-major so the 10-way mean is 9 strided vector adds).
  - Projection reads host-prepacked fp16 feature tiles (HWDGE), writes each
    table slot contiguously; per-table dram tensors let Tile overlap SAGE
    gathers with the tail of projection.
  - SAGE matmuls run feat-major; token-major gathered data is transposed
    via TensorE (identity matmul) in packed PSUM groups.
"""
import os
import numpy as np

F0 = F1 = 10
B = 4096
NCORES = 8
EDGES = B // NCORES          # 512
P = 128
D = 256
NU, NI = 50000, 100000
SEG = 1024                   # hop-2 groups per segment
NSEG = (EDGES * F0) // SEG   # 5
NSEG_A = 2                   # segs 0,1 -> table A; segs 2.. -> table B
CALL = 1024                  # dma_gather idx per call (HW cap)
PROJ_TILE = 512


def _wrap16(a):
    a = np.asarray(a, np.int16)
    w = a.reshape(-1, 16).T
    return np.tile(w, (8, 1)).astype(np.int16)


def _pad512(n):
    return max(512, (int(n) + 511) & ~511)


def _build_plans(inputs):
    """Per-core compacted tables + k-major hop-2 gather indices.

    Returns dict with:
      sizes: dict slot -> padded row count (max over cores)
      rows:  per core: dict slot -> original-table row ids (np.int64 array)
      idx2:  per core: dict side -> [NSEG*F1*CALL] int16 gather idx (k-major)
    """
    h = {}
    for k in ("src_h0", "src_h1", "src_h2", "dst_h0", "dst_h1", "dst_h2"):
        h[k] = np.asarray(inputs[k]).astype(np.int64).reshape(NCORES, -1)

    rows = [dict() for _ in range(NCORES)]
    idx2 = [dict() for _ in range(NCORES)]
    for c in range(NCORES):
        for sd in ("s", "d"):
            pre = "src" if sd == "s" else "dst"
            h0 = h[pre + "_h0"][c]
            h1 = h[pre + "_h1"][c]
            h2 = h[pre + "_h2"][c]
            rows[c][sd + "h0"] = h0
            rows[c][sd + "h1"] = h1
            invs = {}
            for gi, segs in enumerate(_SEG_GROUPS):
                toks = np.concatenate([
                    h2[s * SEG * F1:(s + 1) * SEG * F1] for s in segs])
                u, inv = np.unique(toks, return_inverse=True)
                assert len(u) <= 32000
                rows[c][sd + "ABC"[gi]] = u
                for i, s in enumerate(segs):
                    invs[s] = inv[i * SEG * F1:(i + 1) * SEG * F1]
            calls = []
            for s in range(NSEG):
                inv = invs[s]
                for k in range(F1):
                    # call position q (=token within seg) -> table pos
                    calls.append(inv[np.arange(SEG) * F1 + k])
            idx2[c][sd] = np.concatenate(calls)
    sizes = {}
    for slot in set(_SLOTS):
        sizes[slot] = _pad512(max(len(rows[c][slot]) for c in range(NCORES)))
    return {"sizes": sizes, "rows": rows, "idx2": idx2}


# slot -> (feature kind, bias column) ; src side: h0/h2=user, h1=item
_SLOT_KIND = {"sh0": "u", "sh1": "i", "sA": "u", "sB": "u", "sC": "u",
              "dh0": "i", "dh1": "u", "dA": "i", "dB": "i", "dC": "i"}
# hop-2 seg groups -> tables A/B
_SEG_GROUPS = ((0, 1), (2, 3, 4))
_SEG_TO_TAB = {0: "A", 1: "A", 2: "B", 3: "B", 4: "B"}
# seg-A tables first so their gathers overlap the rest of projection
_SLOTS = ("sA", "dA", "sh1", "dh1", "sh0", "dh0", "sB", "dB")


def _proj_host(feat_u16, feat_i16, plan):
    """Build per-core xt: [512, TOT] fp16, feature-major, tile-permuted.

    Within each 512-col tile, col q = cdim*128 + m holds row m*4 + cdim of
    the slot (so the projected PSUM groups store as contiguous row spans).
    """
    sizes = plan["sizes"]
    xts = []
    for c in range(NCORES):
        parts = []
        for slot in _SLOTS:
            r = plan["rows"][c][slot]
            n = sizes[slot]
            cols = np.zeros((n,), np.int64)
            cols[:len(r)] = r
            perm = cols.reshape(n // PROJ_TILE, P, 4).transpose(
                0, 2, 1).reshape(-1)
            feat = feat_u16 if _SLOT_KIND[slot] == "u" else feat_i16
            parts.append(feat[:, perm])
        xts.append(np.ascontiguousarray(np.concatenate(parts, axis=1)))
    return xts


def _build_bass(plan, debug=False):
    import concourse.tile as tile
    import concourse.bacc as bacc
    from concourse import mybir, library_config
    from concourse.masks import make_identity
    from contextlib import ExitStack

    f16 = mybir.dt.float16
    f32 = mybir.dt.float32
    i16 = mybir.dt.int16
    AF = mybir.ActivationFunctionType

    sizes = plan["sizes"]
    tot = sum(sizes[s] for s in _SLOTS)

    nc = bacc.Bacc("TRN2", target_bir_lowering=False, debug=False,
                   num_devices=NCORES, num_swdge_queues=4)

    xt = nc.dram_tensor("xt", [512, tot], f16, kind="ExternalInput")
    w_pu = nc.dram_tensor("w_pu", [P, 4, D], f16, kind="ExternalInput")
    w_pi = nc.dram_tensor("w_pi", [P, 4, D], f16, kind="ExternalInput")
    b_p = nc.dram_tensor("b_p", [1, 2, 2 * D], f16, kind="ExternalInput")
    wsage = nc.dram_tensor("wsage", [P, 2, 2 * 768], f16, kind="ExternalInput")
    wlin = nc.dram_tensor("wlin", [P, 1], f16, kind="ExternalInput")
    blin = nc.dram_tensor("blin", [1, 1], f32, kind="ExternalInput")
    idx_s = nc.dram_tensor("idx_s", [P, NSEG * F1 * CALL // 16], i16,
                           kind="ExternalInput")
    idx_d = nc.dram_tensor("idx_d", [P, NSEG * F1 * CALL // 16], i16,
                           kind="ExternalInput")
    out = nc.dram_tensor("out", [1, EDGES], f32, kind="ExternalOutput")

    tabs = {s: nc.dram_tensor(f"tab_{s}", [sizes[s], D], f16)
            for s in _SLOTS}

    with tile.TileContext(nc) as tc, ExitStack() as ctx:
        nc.gpsimd.load_library(library_config.mlp)
        tc.strict_bb_all_engine_barrier()

        wpool = ctx.enter_context(tc.tile_pool(name="w", bufs=1))
        w_pu_s = wpool.tile([P, 4, D], f16, tag="wpu")
        w_pi_s = wpool.tile([P, 4, D], f16, tag="wpi")
        b_p_s = wpool.tile([1, 2, 2 * D], f16, tag="bp")
        wsage_s = wpool.tile([P, 2, 2 * 768], f16, tag="wsage")
        wlin_s = wpool.tile([P, 1], f16, tag="wlin")
        blin_s = wpool.tile([1, 1], f32, tag="blin")
        ones_s = wpool.tile([1, P], f16, tag="ones")
        ident = wpool.tile([P, P], f16, tag="ident")
        for dst_, src_ in ((w_pu_s, w_pu), (w_pi_s, w_pi), (b_p_s, b_p),
                           (wsage_s, wsage), (wlin_s, wlin), (blin_s, blin)):
            nc.sync.dma_start(dst_[:], src_[:])
        nc.vector.memset(ones_s[:], 1.0)
        make_identity(nc, ident[:])

        # ---------------- phase A: projection ----------------
        ppool = ctx.enter_context(tc.tile_pool(name="proj", bufs=2))
        pspool = ctx.enter_context(tc.tile_pool(name="pps", bufs=2,
                                                space="PSUM"))
        col_off = 0
        for slot in _SLOTS:
            tab = tabs[slot]
            kind = _SLOT_KIND[slot]
            w_s = w_pu_s if kind == "u" else w_pi_s
            bcol = 0 if kind == "u" else 1
            ntiles = sizes[slot] // PROJ_TILE
            for t in range(ntiles):
                base = col_off + t * PROJ_TILE
                xtt = ppool.tile([P, 4, PROJ_TILE], f16, tag="xtt")
                nc.sync.dma_start(
                    xtt[:],
                    xt[:, base:base + PROJ_TILE].rearrange(
                        "(c p) n -> p c n", p=P))
                sig = ppool.tile([P, 4, D], f16, tag="sig")
                sgl = sig[:].rearrange("p a b -> p (a b)")
                # one PSUM bank per 2 j-groups; bias matmul opens the
                # accumulation group over the whole bank, mains accumulate
                for hb in range(2):
                    ps = pspool.tile([P, 2, D], f32, tag="pps")
                    psl = ps[:].rearrange("p a b -> p (a b)")
                    nc.tensor.matmul(out=psl[:, :], lhsT=ones_s[:, :],
                                     rhs=b_p_s[:, bcol, :], start=True,
                                     stop=False)
                    for jj in range(2):
                        j = 2 * hb + jj
                        for cch in range(4):
                            nc.tensor.matmul(
                                out=ps[:, jj, :],
                                lhsT=xtt[:, cch, j * P:(j + 1) * P],
                                rhs=w_s[:, cch, :], start=False,
                                stop=(cch == 3))
                    nc.scalar.activation(out=sgl[:, hb * 512:(hb + 1) * 512],
                                         in_=psl[:, :], func=AF.Sigmoid)
                nc.scalar.dma_start(
                    tab[t * PROJ_TILE:(t + 1) * PROJ_TILE, :].rearrange(
                        "(p r) d -> p (r d)", r=4),
                    sig[:].rearrange("p r d -> p (r d)"))
            col_off += sizes[slot]

        # ---------------- phase C: SAGE ----------------
        hpool = ctx.enter_context(tc.tile_pool(name="hts", bufs=1))
        # shared scratch: [P,8,256] f32 is the largest shape under this tag
        vpool = ctx.enter_context(tc.tile_pool(name="vtmp", bufs=3))
        plpool = ctx.enter_context(tc.tile_pool(name="pl", bufs=4))
        acpool = ctx.enter_context(tc.tile_pool(name="ac", bufs=2))
        i2pool = ctx.enter_context(tc.tile_pool(name="i2p", bufs=2))
        gpool = ctx.enter_context(tc.tile_pool(name="gts", bufs=1))
        ps2 = ctx.enter_context(tc.tile_pool(name="ps2", bufs=2, space="PSUM"))
        pst = ctx.enter_context(tc.tile_pool(name="pst", bufs=2, space="PSUM"))

        qn = [0]

        def tree10(src, dst, ngr):
            # src [P,2,ngr*10] f16 (col g*10+k) -> dst [P,2,ngr] f32 sums
            t0_t = vpool.tile([P, 2, EDGES], f32, tag="tr")
            t0 = t0_t[:, :, :ngr]
            t1_t = vpool.tile([P, 2, EDGES], f32, tag="tr")
            t1 = t1_t[:, :, :ngr]
            v = src.rearrange("p c (j k) -> p c j k", k=F0)
            nc.vector.tensor_add(t0[:], v[:, :, :, 0], v[:, :, :, 1])
            for i in range(1, 5):
                nc.vector.tensor_add(t1[:], v[:, :, :, 2 * i],
                                     v[:, :, :, 2 * i + 1])
                if i < 4:
                    nc.vector.tensor_add(t0[:], t0[:], t1[:])
            nc.vector.tensor_add(dst, t0[:], t1[:])

        hts = {}
        for si, sd in enumerate(("s", "d")):
            wof = si * 768
            ws0 = wsage_s[:, :, wof:wof + D]
            wa0 = wsage_s[:, :, wof + D:wof + 2 * D]
            ws1 = wsage_s[:, :, wof + 2 * D:wof + 2 * D + 128]
            wa1 = wsage_s[:, :, wof + 2 * D + 128:wof + 768]
            tab_h0 = tabs[sd + "h0"]
            tab_h1 = tabs[sd + "h1"]

            # --- h0T / h1T via transposing DMA loads (token order tables) ---
            h0T = hpool.tile([P, 2, EDGES], f16, tag="h0T")
            h1T = hpool.tile([P, 2, EDGES * F0], f16, tag="h1T")
            for f in range(2):
                nc.sync.dma_start_transpose(
                    h0T[:, f, :], tab_h0[:, f * P:(f + 1) * P])
                nc.sync.dma_start_transpose(
                    h1T[:, f, :], tab_h1[:, f * P:(f + 1) * P])

            # --- n0T = group-sums of h1T ---
            n0f = vpool.tile([P, 2, EDGES], f32, tag="tr")
            tree10(h1T[:], n0f[:], EDGES)
            n0T = hpool.tile([P, 2, EDGES], f16, tag="n0T")
            nc.vector.tensor_copy(n0T[:], n0f[:])

            # --- hop-2 segments -> n1T (rotating k-planes, chained accum) ---
            i2t = i2pool.tile([P, NSEG * F1 * CALL // 16], i16, tag="i2t")
            nc.sync.dma_start(i2t[:], (idx_s if sd == "s" else idx_d)[:])
            n1T = hpool.tile([P, 2, EDGES * F0], f16, tag="n1T")
            for s in range(NSEG):
                tabAB = tabs[sd + _SEG_TO_TAB[s]]
                acc = acpool.tile([P, SEG // P, D], f16, tag="acc")
                planes = []
                for k in range(F1):
                    pl = plpool.tile([P, SEG // P, D], f16, tag="pl")
                    co = (s * F1 + k) * (CALL // 16)
                    nc.gpsimd.dma_gather(
                        pl[:], tabAB[:, :], i2t[:, co:co + CALL // 16],
                        CALL, CALL, D, queue_num=qn[0] % 4)
                    qn[0] += 1
                    planes.append(pl)
                    if k == 1:
                        nc.vector.tensor_add(acc[:], planes[0][:], planes[1][:])
                    elif k > 1:
                        nc.vector.tensor_add(acc[:], acc[:], pl[:])
                n1s = acc
                # transpose to feat-major: 16 tiles packed 4-per-psum
                for f in range(2):
                    for g in range(2):
                        pt = pst.tile([P, 4, P], f16, tag="pt")
                        for b in range(4):
                            nc.tensor.matmul(
                                out=pt[:, b, :],
                                lhsT=n1s[:, g * 4 + b, f * P:(f + 1) * P],
                                rhs=ident[:], is_transpose=True)
                        dst = n1T[:, f, s * SEG + g * 512:
                                  s * SEG + (g + 1) * 512]
                        if (f + g) % 2 == 0:
                            nc.scalar.activation(out=dst, in_=pt[:].rearrange(
                                "p b q -> p (b q)"), func=AF.Copy)
                        else:
                            nc.vector.tensor_copy(dst, pt[:].rearrange(
                                "p b q -> p (b q)"))

            # --- g1 = relu(h1 @ Ws0 + n1 @ Wa0) ---
            g1T = gpool.tile([P, 2, EDGES * F0], f16, tag="g1T")
            for o in range(2):
                for bb in range(EDGES * F0 // 512):
                    sl = slice(bb * 512, (bb + 1) * 512)
                    ps = ps2.tile([P, 512], f32, tag="mmps")
                    for cch in range(2):
                        nc.tensor.matmul(
                            out=ps[:], lhsT=ws0[:, cch, o * P:(o + 1) * P],
                            rhs=h1T[:, cch, sl], start=(cch == 0), stop=False)
                        nc.tensor.matmul(
                            out=ps[:], lhsT=wa0[:, cch, o * P:(o + 1) * P],
                            rhs=n1T[:, cch, sl], start=False, stop=(cch == 1))
                    nc.scalar.activation(out=g1T[:, o, sl], in_=ps[:],
                                         func=AF.Relu)

            # --- nT = group-sums of g1T ---
            nf = vpool.tile([P, 2, EDGES], f32, tag="tr")
            tree10(g1T[:], nf[:], EDGES)
            nT = hpool.tile([P, 2, EDGES], f16, tag="nT")
            nc.vector.tensor_copy(nT[:], nf[:])

            # --- g0 = relu(h0 @ Ws0 + n0 @ Wa0) ---
            g0T = gpool.tile([P, 2, EDGES], f16, tag="g0T")
            for o in range(2):
                ps = ps2.tile([P, 512], f32, tag="mmps")
                for cch in range(2):
                    nc.tensor.matmul(out=ps[:],
                                     lhsT=ws0[:, cch, o * P:(o + 1) * P],
                                     rhs=h0T[:, cch, :], start=(cch == 0),
                                     stop=False)
                    nc.tensor.matmul(out=ps[:],
                                     lhsT=wa0[:, cch, o * P:(o + 1) * P],
                                     rhs=n0T[:, cch, :], start=False,
                                     stop=(cch == 1))
                nc.scalar.activation(out=g0T[:, o, :], in_=ps[:], func=AF.Relu)

            # --- hT = g0 @ Ws1 + n @ Wa1 ---
            ps = ps2.tile([P, 512], f32, tag="mmps")
            for cch in range(2):
                nc.tensor.matmul(out=ps[:], lhsT=ws1[:, cch, :],
                                 rhs=g0T[:, cch, :], start=(cch == 0),
                                 stop=False)
                nc.tensor.matmul(out=ps[:], lhsT=wa1[:, cch, :],
                                 rhs=nT[:, cch, :], start=False,
                                 stop=(cch == 1))
            hT = gpool.tile([P, EDGES], f16, tag=f"hT{sd}")
            nc.scalar.activation(out=hT[:], in_=ps[:], func=AF.Copy)
            hts[sd] = hT

        prod = gpool.tile([P, EDGES], f16, tag="prod")
        nc.vector.tensor_mul(prod[:], hts["s"][:], hts["d"][:])
        psf = ps2.tile([1, EDGES], f32, tag="mmps")
        nc.tensor.matmul(out=psf[:], lhsT=wlin_s[:], rhs=prod[:],
                         start=True, stop=True)
        res = gpool.tile([1, EDGES], f32, tag="res")
        nc.scalar.activation(out=res[:], in_=psf[:], func=AF.Identity,
                             bias=blin_s[:, :1])
        nc.sync.dma_start(out[:], res[:])

    nc.compile()
    return nc


def kernel(**inputs) -> np.ndarray:
    from concourse.bass_utils import run_bass_kernel_spmd

    plan = _build_plans(inputs)

    trace = bool(os.environ.get("GNN_TRACE"))
    if trace:
        import timing_shim
        timing_shim.install()

    nc = _build_bass(plan)

    f16 = np.float16
    # feature tables, feature-major fp16 [512, N]
    feat_u16 = np.ascontiguousarray(
        np.asarray(inputs["user_feat"], np.float32).T).astype(f16)
    feat_i16 = np.ascontiguousarray(
        np.asarray(inputs["item_feat"], np.float32).T).astype(f16)
    xts = _proj_host(feat_u16, feat_i16, plan)

    w_pu = np.ascontiguousarray(
        np.asarray(inputs["W_pu"], np.float32).reshape(4, P, D)
        .transpose(1, 0, 2)).astype(f16)
    w_pi = np.ascontiguousarray(
        np.asarray(inputs["W_pi"], np.float32).reshape(4, P, D)
        .transpose(1, 0, 2)).astype(f16)
    b_p = np.stack([np.tile(np.asarray(inputs["b_pu"], np.float32), 2),
                    np.tile(np.asarray(inputs["b_pi"], np.float32), 2)])[None].astype(f16)

    def sagew(pre):
        s0 = np.asarray(inputs[f"{pre}_self0"], np.float32)
        a0 = np.asarray(inputs[f"{pre}_agg0"], np.float32) * (1.0 / F0)
        s1 = np.asarray(inputs[f"{pre}_self1"], np.float32)
        a1 = np.asarray(inputs[f"{pre}_agg1"], np.float32) * (1.0 / F0)
        cat = np.concatenate([s0, a0, s1, a1], axis=1)  # [256, 768]
        return cat.reshape(2, P, 768).transpose(1, 0, 2)

    wsage = np.ascontiguousarray(
        np.concatenate([sagew("u"), sagew("i")], axis=2)).astype(f16)
    wlin = np.asarray(inputs["W_lin"], np.float32).astype(f16)
    blin = np.asarray(inputs["b_lin"], np.float32).reshape(1, 1)

    in_maps = []
    for c in range(NCORES):
        m = {
            "xt": xts[c], "w_pu": w_pu, "w_pi": w_pi, "b_p": b_p,
            "wsage": wsage, "wlin": wlin, "blin": blin,
            "idx_s": _wrap16(plan["idx2"][c]["s"]),
            "idx_d": _wrap16(plan["idx2"][c]["d"]),
        }
        in_maps.append(m)

    kw = dict(trace=True, trace_cores=list(range(NCORES))) if trace else {}
    res = run_bass_kernel_spmd(nc, in_maps, core_ids=list(range(NCORES)), **kw)
    if trace and res.exec_time_ns:
        print(f"HW exec time: {res.exec_time_ns} ns")
        kernel.last_exec_ns = res.exec_time_ns

    logits = np.concatenate([res.results[c]["out"][0] for c in range(NCORES)])
    return logits.reshape(B, 1).astype(np.float32)


# revision 24
# speedup vs baseline: 1.2104x; 1.1877x over previous
"""GraphSAGE (2-layer, mean-agg) edge-scoring kernel for 8 trn2 NeuronCores.

Per-core private compacted tables (no pair sharing, no barrier):
  - h0/h1 hop tables stored in token order (plain / transposing DMA loads).
  - hop-2 rows deduped into 2 tables per side (segs {0,1} and {2,3,4}),
    each guaranteed <= 30720 rows so plain positive int16 dma_gather idx work
    with no chunking and no reorder (gathers are order-preserving,
    1024-idx calls, k-major so the 10-way mean is 9 strided vector adds).
  - Projection reads host-prepacked fp16 feature tiles (HWDGE), writes each
    table slot contiguously; per-table dram tensors let Tile overlap SAGE
    gathers with the tail of projection.
  - SAGE matmuls run feat-major; token-major gathered data is transposed
    via TensorE (identity matmul) in packed PSUM groups.
"""
import os
import numpy as np

F0 = F1 = 10
B = 4096
NCORES = 8
EDGES = B // NCORES          # 512
P = 128
D = 256
NU, NI = 50000, 100000
SEG = 1024                   # hop-2 groups per segment
NSEG = (EDGES * F0) // SEG   # 5
NSEG_A = 2                   # segs 0,1 -> table A; segs 2.. -> table B
CALL = 1024                  # dma_gather idx per call (HW cap)
PROJ_TILE = 512


def _wrap16(a):
    a = np.asarray(a, np.int16)
    w = a.reshape(-1, 16).T
    return np.tile(w, (8, 1)).astype(np.int16)


def _pad512(n):
    return max(512, (int(n) + 511) & ~511)


def _build_plans(inputs):
    """Per-core compacted tables + k-major hop-2 gather indices.

    Returns dict with:
      sizes: dict slot -> padded row count (max over cores)
      rows:  per core: dict slot -> original-table row ids (np.int64 array)
      idx2:  per core: dict side -> [NSEG*F1*CALL] int16 gather idx (k-major)
    """
    h = {}
    for k in ("src_h0", "src_h1", "src_h2", "dst_h0", "dst_h1", "dst_h2"):
        h[k] = np.asarray(inputs[k]).astype(np.int64).reshape(NCORES, -1)

    rows = [dict() for _ in range(NCORES)]
    idx2 = [dict() for _ in range(NCORES)]
    for c in range(NCORES):
        for sd in ("s", "d"):
            pre = "src" if sd == "s" else "dst"
            h0 = h[pre + "_h0"][c]
            h1 = h[pre + "_h1"][c]
            h2 = h[pre + "_h2"][c]
            rows[c][sd + "h0"] = h0
            rows[c][sd + "h1"] = h1
            invs = {}
            for gi, segs in enumerate(_SEG_GROUPS):
                toks = np.concatenate([
                    h2[s * SEG * F1:(s + 1) * SEG * F1] for s in segs])
                u, inv = np.unique(toks, return_inverse=True)
                assert len(u) <= 32000
                rows[c][sd + "ABC"[gi]] = u
                for i, s in enumerate(segs):
                    invs[s] = inv[i * SEG * F1:(i + 1) * SEG * F1]
            calls = []
            for s in range(NSEG):
                inv = invs[s]
                for k in range(F1):
                    # call position q (=token within seg) -> table pos
                    calls.append(inv[np.arange(SEG) * F1 + k])
            idx2[c][sd] = np.concatenate(calls)
    sizes = {}
    for slot in set(_SLOTS):
        sizes[slot] = _pad512(max(len(rows[c][slot]) for c in range(NCORES)))
    return {"sizes": sizes, "rows": rows, "idx2": idx2}


# slot -> (feature kind, bias column) ; src side: h0/h2=user, h1=item
_SLOT_KIND = {"sh0": "u", "sh1": "i", "sA": "u", "sB": "u", "sC": "u",
              "dh0": "i", "dh1": "u", "dA": "i", "dB": "i", "dC": "i"}
# hop-2 seg groups -> tables A/B
_SEG_GROUPS = ((0, 1), (2, 3, 4))
_SEG_TO_TAB = {0: "A", 1: "A", 2: "B", 3: "B", 4: "B"}
# seg-A tables first so their gathers overlap the rest of projection
_SLOTS = ("sA", "dA", "sh1", "dh1", "sh0", "dh0", "sB", "dB")


def _proj_host(feat_u16, feat_i16, plan):
    """Build per-core xt: [512, TOT] fp16, feature-major, tile-permuted.

    Within each 512-col tile, col q = cdim*128 + m holds row m*4 + cdim of
    the slot (so the projected PSUM groups store as contiguous row spans).
    """
    sizes = plan["sizes"]
    xts = []
    for c in range(NCORES):
        parts = []
        for slot in _SLOTS:
            r = plan["rows"][c][slot]
            n = sizes[slot]
            cols = np.zeros((n,), np.int64)
            cols[:len(r)] = r
            perm = cols.reshape(n // PROJ_TILE, P, 4).transpose(
                0, 2, 1).reshape(-1)
            feat = feat_u16 if _SLOT_KIND[slot] == "u" else feat_i16
            parts.append(feat[:, perm])
        xts.append(np.ascontiguousarray(np.concatenate(parts, axis=1)))
    return xts


def _build_bass(plan, debug=False):
    import concourse.tile as tile
    import concourse.bacc as bacc
    from concourse import mybir, library_config
    from concourse.masks import make_identity
    from contextlib import ExitStack

    f16 = mybir.dt.float16
    f32 = mybir.dt.float32
    i16 = mybir.dt.int16
    AF = mybir.ActivationFunctionType

    sizes = plan["sizes"]
    tot = sum(sizes[s] for s in _SLOTS)

    nc = bacc.Bacc("TRN2", target_bir_lowering=False, debug=False,
                   num_devices=NCORES, num_swdge_queues=4)

    xt = nc.dram_tensor("xt", [512, tot], f16, kind="ExternalInput")
    w_pu = nc.dram_tensor("w_pu", [P, 4, D], f16, kind="ExternalInput")
    w_pi = nc.dram_tensor("w_pi", [P, 4, D], f16, kind="ExternalInput")
    b_p = nc.dram_tensor("b_p", [1, 2, 2 * D], f16, kind="ExternalInput")
    wsage = nc.dram_tensor("wsage", [P, 2, 2 * 768], f16, kind="ExternalInput")
    wlin = nc.dram_tensor("wlin", [P, 1], f16, kind="ExternalInput")
    blin = nc.dram_tensor("blin", [1, 1], f32, kind="ExternalInput")
    idx_s = nc.dram_tensor("idx_s", [P, NSEG * F1 * CALL // 16], i16,
                           kind="ExternalInput")
    idx_d = nc.dram_tensor("idx_d", [P, NSEG * F1 * CALL // 16], i16,
                           kind="ExternalInput")
    out = nc.dram_tensor("out", [1, EDGES], f32, kind="ExternalOutput")

    tabs = {s: nc.dram_tensor(f"tab_{s}", [sizes[s], D], f16)
            for s in _SLOTS}

    with tile.TileContext(nc) as tc, ExitStack() as ctx:
        nc.gpsimd.load_library(library_config.mlp)
        tc.strict_bb_all_engine_barrier()

        wpool = ctx.enter_context(tc.tile_pool(name="w", bufs=1))
        w_pu_s = wpool.tile([P, 4, D], f16, tag="wpu")
        w_pi_s = wpool.tile([P, 4, D], f16, tag="wpi")
        b_p_s = wpool.tile([1, 2, 2 * D], f16, tag="bp")
        wsage_s = wpool.tile([P, 2, 2 * 768], f16, tag="wsage")
        wlin_s = wpool.tile([P, 1], f16, tag="wlin")
        blin_s = wpool.tile([1, 1], f32, tag="blin")
        ones_s = wpool.tile([1, P], f16, tag="ones")
        ident = wpool.tile([P, P], f16, tag="ident")
        for dst_, src_ in ((w_pu_s, w_pu), (w_pi_s, w_pi), (b_p_s, b_p),
                           (wsage_s, wsage), (wlin_s, wlin), (blin_s, blin)):
            nc.sync.dma_start(dst_[:], src_[:])
        nc.vector.memset(ones_s[:], 1.0)
        make_identity(nc, ident[:])

        # ---------------- phase A: projection ----------------
        ppool = ctx.enter_context(tc.tile_pool(name="proj", bufs=3))
        pspool = ctx.enter_context(tc.tile_pool(name="pps", bufs=3,
                                                space="PSUM"))
        col_off = 0
        for slot in _SLOTS:
            tab = tabs[slot]
            kind = _SLOT_KIND[slot]
            w_s = w_pu_s if kind == "u" else w_pi_s
            bcol = 0 if kind == "u" else 1
            ntiles = sizes[slot] // PROJ_TILE
            for t in range(ntiles):
                base = col_off + t * PROJ_TILE
                xtt = ppool.tile([P, 4, PROJ_TILE], f16, tag="xtt")
                nc.sync.dma_start(
                    xtt[:],
                    xt[:, base:base + PROJ_TILE].rearrange(
                        "(c p) n -> p c n", p=P))
                sig = ppool.tile([P, 4, D], f16, tag="sig")
                sgl = sig[:].rearrange("p a b -> p (a b)")
                # one PSUM bank per 2 j-groups; bias matmul opens the
                # accumulation group over the whole bank, mains accumulate
                for hb in range(2):
                    ps = pspool.tile([P, 2, D], f32, tag="pps")
                    psl = ps[:].rearrange("p a b -> p (a b)")
                    nc.tensor.matmul(out=psl[:, :], lhsT=ones_s[:, :],
                                     rhs=b_p_s[:, bcol, :], start=True,
                                     stop=False)
                    for jj in range(2):
                        j = 2 * hb + jj
                        for cch in range(4):
                            nc.tensor.matmul(
                                out=ps[:, jj, :],
                                lhsT=xtt[:, cch, j * P:(j + 1) * P],
                                rhs=w_s[:, cch, :], start=False,
                                stop=(cch == 3))
                    nc.scalar.activation(out=sgl[:, hb * 512:(hb + 1) * 512],
                                         in_=psl[:, :], func=AF.Sigmoid)
                nc.scalar.dma_start(
                    tab[t * PROJ_TILE:(t + 1) * PROJ_TILE, :].rearrange(
                        "(p r) d -> p (r d)", r=4),
                    sig[:].rearrange("p r d -> p (r d)"))
            col_off += sizes[slot]

        # ---------------- phase C: SAGE ----------------
        hpool = ctx.enter_context(tc.tile_pool(name="hts", bufs=1))
        # shared scratch: [P,8,256] f32 is the largest shape under this tag
        vpool = ctx.enter_context(tc.tile_pool(name="vtmp", bufs=3))
        plpool = ctx.enter_context(tc.tile_pool(name="pl", bufs=6))
        acpool = ctx.enter_context(tc.tile_pool(name="ac", bufs=3))
        i2pool = ctx.enter_context(tc.tile_pool(name="i2p", bufs=2))
        gpool = ctx.enter_context(tc.tile_pool(name="gts", bufs=1))
        ps2 = ctx.enter_context(tc.tile_pool(name="ps2", bufs=2, space="PSUM"))
        pst = ctx.enter_context(tc.tile_pool(name="pst", bufs=2, space="PSUM"))

        qn = [0]

        def tree10(src, dst, ngr):
            # src [P,2,ngr*10] f16 (col g*10+k) -> dst [P,2,ngr] f32 sums
            t0_t = vpool.tile([P, 2, EDGES], f32, tag="tr")
            t0 = t0_t[:, :, :ngr]
            t1_t = vpool.tile([P, 2, EDGES], f32, tag="tr")
            t1 = t1_t[:, :, :ngr]
            v = src.rearrange("p c (j k) -> p c j k", k=F0)
            nc.vector.tensor_add(t0[:], v[:, :, :, 0], v[:, :, :, 1])
            for i in range(1, 5):
                nc.vector.tensor_add(t1[:], v[:, :, :, 2 * i],
                                     v[:, :, :, 2 * i + 1])
                if i < 4:
                    nc.vector.tensor_add(t0[:], t0[:], t1[:])
            nc.vector.tensor_add(dst, t0[:], t1[:])

        hts = {}
        for si, sd in enumerate(("s", "d")):
            wof = si * 768
            ws0 = wsage_s[:, :, wof:wof + D]
            wa0 = wsage_s[:, :, wof + D:wof + 2 * D]
            ws1 = wsage_s[:, :, wof + 2 * D:wof + 2 * D + 128]
            wa1 = wsage_s[:, :, wof + 2 * D + 128:wof + 768]
            tab_h0 = tabs[sd + "h0"]
            tab_h1 = tabs[sd + "h1"]

            # --- h0T / h1T via transposing DMA loads (token order tables) ---
            h0T = hpool.tile([P, 2, EDGES], f16, tag="h0T")
            h1T = hpool.tile([P, 2, EDGES * F0], f16, tag="h1T")
            for f in range(2):
                nc.sync.dma_start_transpose(
                    h0T[:, f, :], tab_h0[:, f * P:(f + 1) * P])
                nc.sync.dma_start_transpose(
                    h1T[:, f, :], tab_h1[:, f * P:(f + 1) * P])

            # --- n0T = group-sums of h1T ---
            n0f = vpool.tile([P, 2, EDGES], f32, tag="tr")
            tree10(h1T[:], n0f[:], EDGES)
            n0T = hpool.tile([P, 2, EDGES], f16, tag="n0T")
            nc.vector.tensor_copy(n0T[:], n0f[:])

            # --- hop-2 segments -> n1T (rotating k-planes, chained accum) ---
            i2t = i2pool.tile([P, NSEG * F1 * CALL // 16], i16, tag="i2t")
            nc.sync.dma_start(i2t[:], (idx_s if sd == "s" else idx_d)[:])
            n1T = hpool.tile([P, 2, EDGES * F0], f16, tag="n1T")
            for s in range(NSEG):
                tabAB = tabs[sd + _SEG_TO_TAB[s]]
                acc = acpool.tile([P, SEG // P, D], f16, tag="acc")
                planes = []
                for k in range(F1):
                    pl = plpool.tile([P, SEG // P, D], f16, tag="pl")
                    co = (s * F1 + k) * (CALL // 16)
                    nc.gpsimd.dma_gather(
                        pl[:], tabAB[:, :], i2t[:, co:co + CALL // 16],
                        CALL, CALL, D, queue_num=qn[0] % 4)
                    qn[0] += 1
                    planes.append(pl)
                    if k == 1:
                        nc.vector.tensor_add(acc[:], planes[0][:], planes[1][:])
                    elif k > 1:
                        nc.vector.tensor_add(acc[:], acc[:], pl[:])
                n1s = acc
                # transpose to feat-major: 16 tiles packed 4-per-psum
                for f in range(2):
                    for g in range(2):
                        pt = pst.tile([P, 4, P], f16, tag="pt")
                        for b in range(4):
                            nc.tensor.matmul(
                                out=pt[:, b, :],
                                lhsT=n1s[:, g * 4 + b, f * P:(f + 1) * P],
                                rhs=ident[:], is_transpose=True)
                        dst = n1T[:, f, s * SEG + g * 512:
                                  s * SEG + (g + 1) * 512]
                        if (f + g) % 2 == 0:
                            nc.scalar.activation(out=dst, in_=pt[:].rearrange(
                                "p b q -> p (b q)"), func=AF.Copy)
                        else:
                            nc.vector.tensor_copy(dst, pt[:].rearrange(
                                "p b q -> p (b q)"))

            # --- g1 = relu(h1 @ Ws0 + n1 @ Wa0) ---
            g1T = gpool.tile([P, 2, EDGES * F0], f16, tag="g1T")
            for o in range(2):
                for bb in range(EDGES * F0 // 512):
                    sl = slice(bb * 512, (bb + 1) * 512)
                    ps = ps2.tile([P, 512], f32, tag="mmps")
                    for cch in range(2):
                        nc.tensor.matmul(
                            out=ps[:], lhsT=ws0[:, cch, o * P:(o + 1) * P],
                            rhs=h1T[:, cch, sl], start=(cch == 0), stop=False)
                        nc.tensor.matmul(
                            out=ps[:], lhsT=wa0[:, cch, o * P:(o + 1) * P],
                            rhs=n1T[:, cch, sl], start=False, stop=(cch == 1))
                    nc.scalar.activation(out=g1T[:, o, sl], in_=ps[:],
                                         func=AF.Relu)

            # --- nT = group-sums of g1T ---
            nf = vpool.tile([P, 2, EDGES], f32, tag="tr")
            tree10(g1T[:], nf[:], EDGES)
            nT = hpool.tile([P, 2, EDGES], f16, tag="nT")
            nc.vector.tensor_copy(nT[:], nf[:])

            # --- g0 = relu(h0 @ Ws0 + n0 @ Wa0) ---
            g0T = gpool.tile([P, 2, EDGES], f16, tag="g0T")
            for o in range(2):
                ps = ps2.tile([P, 512], f32, tag="mmps")
                for cch in range(2):
                    nc.tensor.matmul(out=ps[:],
                                     lhsT=ws0[:, cch, o * P:(o + 1) * P],
                                     rhs=h0T[:, cch, :], start=(cch == 0),
                                     stop=False)
                    nc.tensor.matmul(out=ps[:],
                                     lhsT=wa0[:, cch, o * P:(o + 1) * P],
                                     rhs=n0T[:, cch, :], start=False,
                                     stop=(cch == 1))
                nc.scalar.activation(out=g0T[:, o, :], in_=ps[:], func=AF.Relu)

            # --- hT = g0 @ Ws1 + n @ Wa1 ---
            ps = ps2.tile([P, 512], f32, tag="mmps")
            for cch in range(2):
                nc.tensor.matmul(out=ps[:], lhsT=ws1[:, cch, :],
                                 rhs=g0T[:, cch, :], start=(cch == 0),
                                 stop=False)
                nc.tensor.matmul(out=ps[:], lhsT=wa1[:, cch, :],
                                 rhs=nT[:, cch, :], start=False,
                                 stop=(cch == 1))
            hT = gpool.tile([P, EDGES], f16, tag=f"hT{sd}")
            nc.scalar.activation(out=hT[:], in_=ps[:], func=AF.Copy)
            hts[sd] = hT

        prod = gpool.tile([P, EDGES], f16, tag="prod")
        nc.vector.tensor_mul(prod[:], hts["s"][:], hts["d"][:])
        psf = ps2.tile([1, EDGES], f32, tag="mmps")
        nc.tensor.matmul(out=psf[:], lhsT=wlin_s[:], rhs=prod[:],
                         start=True, stop=True)
        res = gpool.tile([1, EDGES], f32, tag="res")
        nc.scalar.activation(out=res[:], in_=psf[:], func=AF.Identity,
                             bias=blin_s[:, :1])
        nc.sync.dma_start(out[:], res[:])

    nc.compile()
    return nc


def kernel(**inputs) -> np.ndarray:
    from concourse.bass_utils import run_bass_kernel_spmd

    plan = _build_plans(inputs)

    trace = bool(os.environ.get("GNN_TRACE"))
    if trace:
        import timing_shim
        timing_shim.install()

    nc = _build_bass(plan)

    f16 = np.float16
    # feature tables, feature-major fp16 [512, N]
    feat_u16 = np.ascontiguousarray(
        np.asarray(inputs["user_feat"], np.float32).T).astype(f16)
    feat_i16 = np.ascontiguousarray(
        np.asarray(inputs["item_feat"], np.float32).T).astype(f16)
    xts = _proj_host(feat_u16, feat_i16, plan)

    w_pu = np.ascontiguousarray(
        np.asarray(inputs["W_pu"], np.float32).reshape(4, P, D)
        .transpose(1, 0, 2)).astype(f16)
    w_pi = np.ascontiguousarray(
        np.asarray(inputs["W_pi"], np.float32).reshape(4, P, D)
        .transpose(1, 0, 2)).astype(f16)
    b_p = np.stack([np.tile(np.asarray(inputs["b_pu"], np.float32), 2),
                    np.tile(np.asarray(inputs["b_pi"], np.float32), 2)])[None].astype(f16)

    def sagew(pre):
        s0 = np.asarray(inputs[f"{pre}_self0"], np.float32)
        a0 = np.asarray(inputs[f"{pre}_agg0"], np.float32) * (1.0 / F0)
        s1 = np.asarray(inputs[f"{pre}_self1"], np.float32)
        a1 = np.asarray(inputs[f"{pre}_agg1"], np.float32) * (1.0 / F0)
        cat = np.concatenate([s0, a0, s1, a1], axis=1)  # [256, 768]
        return cat.reshape(2, P, 768).transpose(1, 0, 2)

    wsage = np.ascontiguousarray(
        np.concatenate([sagew("u"), sagew("i")], axis=2)).astype(f16)
    wlin = np.asarray(inputs["W_lin"], np.float32).astype(f16)
    blin = np.asarray(inputs["b_lin"], np.float32).reshape(1, 1)

    in_maps = []
    for c in range(NCORES):
        m = {
            "xt": xts[c], "w_pu": w_pu, "w_pi": w_pi, "b_p": b_p,
            "wsage": wsage, "wlin": wlin, "blin": blin,
            "idx_s": _wrap16(plan["idx2"][c]["s"]),
            "idx_d": _wrap16(plan["idx2"][c]["d"]),
        }
        in_maps.append(m)

    kw = dict(trace=True, trace_cores=list(range(NCORES))) if trace else {}
    res = run_bass_kernel_spmd(nc, in_maps, core_ids=list(range(NCORES)), **kw)
    if trace and res.exec_time_ns:
        print(f"HW exec time: {res.exec_time_ns} ns")
        kernel.last_exec_ns = res.exec_time_ns

    logits = np.concatenate([res.results[c]["out"][0] for c in range(NCORES)])
    return logits.reshape(B, 1).astype(np.float32)


# revision 25
# speedup vs baseline: 1.2890x; 1.0649x over previous
"""GraphSAGE (2-layer, mean-agg) edge-scoring kernel for 8 trn2 NeuronCores.

Per-core private compacted tables (no pair sharing, no barrier):
  - h0/h1 hop tables stored in token order (plain / transposing DMA loads).
  - hop-2 rows deduped into 2 tables per side (segs {0,1} and {2,3,4}),
    each guaranteed <= 30720 rows so plain positive int16 dma_gather idx work
    with no chunking and no reorder (gathers are order-preserving,
    1024-idx calls, k-major so the 10-way mean is 9 strided vector adds).
  - Projection reads host-prepacked fp16 feature tiles (HWDGE), writes each
    table slot contiguously; per-table dram tensors let Tile overlap SAGE
    gathers with the tail of projection.
  - SAGE matmuls run feat-major; token-major gathered data is transposed
    via TensorE (identity matmul) in packed PSUM groups.
"""
import os
import numpy as np

F0 = F1 = 10
B = 4096
NCORES = 8
EDGES = B // NCORES          # 512
P = 128
D = 256
NU, NI = 50000, 100000
SEG = 1024                   # hop-2 groups per segment
NSEG = (EDGES * F0) // SEG   # 5
NSEG_A = 2                   # segs 0,1 -> table A; segs 2.. -> table B
CALL = 1024                  # dma_gather idx per call (HW cap)
PROJ_TILE = 512


def _wrap16(a):
    a = np.asarray(a, np.int16)
    w = a.reshape(-1, 16).T
    return np.tile(w, (8, 1)).astype(np.int16)


def _pad512(n):
    return max(512, (int(n) + 511) & ~511)


def _build_plans(inputs):
    """Per-core compacted tables + k-major hop-2 gather indices.

    Returns dict with:
      sizes: dict slot -> padded row count (max over cores)
      rows:  per core: dict slot -> original-table row ids (np.int64 array)
      idx2:  per core: dict side -> [NSEG*F1*CALL] int16 gather idx (k-major)
    """
    h = {}
    for k in ("src_h0", "src_h1", "src_h2", "dst_h0", "dst_h1", "dst_h2"):
        h[k] = np.asarray(inputs[k]).astype(np.int64).reshape(NCORES, -1)

    rows = [dict() for _ in range(NCORES)]
    idx2 = [dict() for _ in range(NCORES)]
    for c in range(NCORES):
        for sd in ("s", "d"):
            pre = "src" if sd == "s" else "dst"
            h0 = h[pre + "_h0"][c]
            h1 = h[pre + "_h1"][c]
            h2 = h[pre + "_h2"][c]
            rows[c][sd + "h0"] = h0
            rows[c][sd + "h1"] = h1
            invs = {}
            for gi, segs in enumerate(_SEG_GROUPS):
                toks = np.concatenate([
                    h2[s * SEG * F1:(s + 1) * SEG * F1] for s in segs])
                u, inv = np.unique(toks, return_inverse=True)
                assert len(u) <= 32000
                rows[c][sd + "ABC"[gi]] = u
                for i, s in enumerate(segs):
                    invs[s] = inv[i * SEG * F1:(i + 1) * SEG * F1]
            calls = []
            for s in range(NSEG):
                inv = invs[s]
                for k in range(F1):
                    # call position q (=token within seg) -> table pos
                    calls.append(inv[np.arange(SEG) * F1 + k])
            idx2[c][sd] = np.concatenate(calls)
    sizes = {}
    for slot in set(_SLOTS):
        sizes[slot] = _pad512(max(len(rows[c][slot]) for c in range(NCORES)))
    return {"sizes": sizes, "rows": rows, "idx2": idx2}


# slot -> (feature kind, bias column) ; src side: h0/h2=user, h1=item
_SLOT_KIND = {"sh0": "u", "sh1": "i", "sA": "u", "sB": "u", "sC": "u",
              "dh0": "i", "dh1": "u", "dA": "i", "dB": "i", "dC": "i"}
# hop-2 seg groups -> tables A/B
_SEG_GROUPS = ((0, 1), (2, 3, 4))
_SEG_TO_TAB = {0: "A", 1: "A", 2: "B", 3: "B", 4: "B"}
# seg-A tables first so their gathers overlap the rest of projection
_SLOTS = ("sA", "dA", "sh1", "dh1", "sh0", "dh0", "sB", "dB")


def _proj_host(feat_u16, feat_i16, plan):
    """Build per-core xt: [512, TOT] fp16, feature-major, tile-permuted.

    Within each 512-col tile, col q = cdim*128 + m holds row m*4 + cdim of
    the slot (so the projected PSUM groups store as contiguous row spans).
    """
    sizes = plan["sizes"]
    xts = []
    for c in range(NCORES):
        parts = []
        for slot in _SLOTS:
            r = plan["rows"][c][slot]
            n = sizes[slot]
            cols = np.zeros((n,), np.int64)
            cols[:len(r)] = r
            perm = cols.reshape(n // PROJ_TILE, P, 4).transpose(
                0, 2, 1).reshape(-1)
            feat = feat_u16 if _SLOT_KIND[slot] == "u" else feat_i16
            parts.append(feat[:, perm])
        xts.append(np.ascontiguousarray(np.concatenate(parts, axis=1)))
    return xts


def _build_bass(plan, debug=False):
    import concourse.tile as tile
    import concourse.bacc as bacc
    from concourse import mybir, library_config
    from concourse.masks import make_identity
    from contextlib import ExitStack

    f16 = mybir.dt.float16
    f32 = mybir.dt.float32
    i16 = mybir.dt.int16
    AF = mybir.ActivationFunctionType

    sizes = plan["sizes"]
    tot = sum(sizes[s] for s in _SLOTS)

    nc = bacc.Bacc("TRN2", target_bir_lowering=False, debug=False,
                   num_devices=NCORES, num_swdge_queues=4)

    xt = nc.dram_tensor("xt", [512, tot], f16, kind="ExternalInput")
    w_pu = nc.dram_tensor("w_pu", [P, 4, D], f16, kind="ExternalInput")
    w_pi = nc.dram_tensor("w_pi", [P, 4, D], f16, kind="ExternalInput")
    b_p = nc.dram_tensor("b_p", [1, 2, 2 * D], f16, kind="ExternalInput")
    wsage = nc.dram_tensor("wsage", [P, 2, 2 * 768], f16, kind="ExternalInput")
    wlin = nc.dram_tensor("wlin", [P, 1], f16, kind="ExternalInput")
    blin = nc.dram_tensor("blin", [1, 1], f32, kind="ExternalInput")
    idx_s = nc.dram_tensor("idx_s", [P, NSEG * F1 * CALL // 16], i16,
                           kind="ExternalInput")
    idx_d = nc.dram_tensor("idx_d", [P, NSEG * F1 * CALL // 16], i16,
                           kind="ExternalInput")
    out = nc.dram_tensor("out", [1, EDGES], f32, kind="ExternalOutput")

    tabs = {s: nc.dram_tensor(f"tab_{s}", [sizes[s], D], f16)
            for s in _SLOTS}

    with tile.TileContext(nc) as tc, ExitStack() as ctx:
        nc.gpsimd.load_library(library_config.mlp)
        tc.strict_bb_all_engine_barrier()

        wpool = ctx.enter_context(tc.tile_pool(name="w", bufs=1))
        w_pu_s = wpool.tile([P, 4, D], f16, tag="wpu")
        w_pi_s = wpool.tile([P, 4, D], f16, tag="wpi")
        b_p_s = wpool.tile([1, 2, 2 * D], f16, tag="bp")
        wsage_s = wpool.tile([P, 2, 2 * 768], f16, tag="wsage")
        wlin_s = wpool.tile([P, 1], f16, tag="wlin")
        blin_s = wpool.tile([1, 1], f32, tag="blin")
        ones_s = wpool.tile([1, P], f16, tag="ones")
        ident = wpool.tile([P, P], f16, tag="ident")
        for dst_, src_ in ((w_pu_s, w_pu), (w_pi_s, w_pi), (b_p_s, b_p),
                           (wsage_s, wsage), (wlin_s, wlin), (blin_s, blin)):
            nc.sync.dma_start(dst_[:], src_[:])
        nc.vector.memset(ones_s[:], 1.0)
        make_identity(nc, ident[:])

        # ---------------- phase A: projection ----------------
        ppool = ctx.enter_context(tc.tile_pool(name="proj", bufs=3))
        pspool = ctx.enter_context(tc.tile_pool(name="pps", bufs=4,
                                                space="PSUM"))
        col_off = 0
        for slot in _SLOTS:
            tab = tabs[slot]
            kind = _SLOT_KIND[slot]
            w_s = w_pu_s if kind == "u" else w_pi_s
            bcol = 0 if kind == "u" else 1
            ntiles = sizes[slot] // PROJ_TILE
            for t in range(ntiles):
                base = col_off + t * PROJ_TILE
                xtt = ppool.tile([P, 4, PROJ_TILE], f16, tag="xtt")
                nc.sync.dma_start(
                    xtt[:],
                    xt[:, base:base + PROJ_TILE].rearrange(
                        "(c p) n -> p c n", p=P))
                sig = ppool.tile([P, 4, D], f16, tag="sig")
                sgl = sig[:].rearrange("p a b -> p (a b)")
                # one PSUM bank per 2 j-groups; bias matmul opens the
                # accumulation group over the whole bank, mains accumulate
                for hb in range(2):
                    ps = pspool.tile([P, 2, D], f32, tag="pps")
                    psl = ps[:].rearrange("p a b -> p (a b)")
                    nc.tensor.matmul(out=psl[:, :], lhsT=ones_s[:, :],
                                     rhs=b_p_s[:, bcol, :], start=True,
                                     stop=False)
                    for jj in range(2):
                        j = 2 * hb + jj
                        for cch in range(4):
                            nc.tensor.matmul(
                                out=ps[:, jj, :],
                                lhsT=xtt[:, cch, j * P:(j + 1) * P],
                                rhs=w_s[:, cch, :], start=False,
                                stop=(cch == 3))
                    nc.scalar.activation(out=sgl[:, hb * 512:(hb + 1) * 512],
                                         in_=psl[:, :], func=AF.Sigmoid)
                nc.scalar.dma_start(
                    tab[t * PROJ_TILE:(t + 1) * PROJ_TILE, :].rearrange(
                        "(p r) d -> p (r d)", r=4),
                    sig[:].rearrange("p r d -> p (r d)"))
            col_off += sizes[slot]

        # ---------------- phase C: SAGE ----------------
        hpool = ctx.enter_context(tc.tile_pool(name="hts", bufs=1))
        # shared scratch: [P,8,256] f32 is the largest shape under this tag
        vpool = ctx.enter_context(tc.tile_pool(name="vtmp", bufs=3))
        plpool = ctx.enter_context(tc.tile_pool(name="pl", bufs=8))
        acpool = ctx.enter_context(tc.tile_pool(name="ac", bufs=4))
        i2pool = ctx.enter_context(tc.tile_pool(name="i2p", bufs=2))
        gpool = ctx.enter_context(tc.tile_pool(name="gts", bufs=1))
        ps2 = ctx.enter_context(tc.tile_pool(name="ps2", bufs=2, space="PSUM"))
        pst = ctx.enter_context(tc.tile_pool(name="pst", bufs=2, space="PSUM"))

        qn = [0]

        def tree10(src, dst, ngr):
            # src [P,2,ngr*10] f16 (col g*10+k) -> dst [P,2,ngr] f32 sums
            t0_t = vpool.tile([P, 2, EDGES], f32, tag="tr")
            t0 = t0_t[:, :, :ngr]
            t1_t = vpool.tile([P, 2, EDGES], f32, tag="tr")
            t1 = t1_t[:, :, :ngr]
            v = src.rearrange("p c (j k) -> p c j k", k=F0)
            nc.vector.tensor_add(t0[:], v[:, :, :, 0], v[:, :, :, 1])
            for i in range(1, 5):
                nc.vector.tensor_add(t1[:], v[:, :, :, 2 * i],
                                     v[:, :, :, 2 * i + 1])
                if i < 4:
                    nc.vector.tensor_add(t0[:], t0[:], t1[:])
            nc.vector.tensor_add(dst, t0[:], t1[:])

        hts = {}
        for si, sd in enumerate(("s", "d")):
            wof = si * 768
            ws0 = wsage_s[:, :, wof:wof + D]
            wa0 = wsage_s[:, :, wof + D:wof + 2 * D]
            ws1 = wsage_s[:, :, wof + 2 * D:wof + 2 * D + 128]
            wa1 = wsage_s[:, :, wof + 2 * D + 128:wof + 768]
            tab_h0 = tabs[sd + "h0"]
            tab_h1 = tabs[sd + "h1"]

            # --- h0T / h1T via transposing DMA loads (token order tables) ---
            h0T = hpool.tile([P, 2, EDGES], f16, tag="h0T")
            h1T = hpool.tile([P, 2, EDGES * F0], f16, tag="h1T")
            for f in range(2):
                nc.sync.dma_start_transpose(
                    h0T[:, f, :], tab_h0[:, f * P:(f + 1) * P])
                nc.sync.dma_start_transpose(
                    h1T[:, f, :], tab_h1[:, f * P:(f + 1) * P])

            # --- n0T = group-sums of h1T ---
            n0f = vpool.tile([P, 2, EDGES], f32, tag="tr")
            tree10(h1T[:], n0f[:], EDGES)
            n0T = hpool.tile([P, 2, EDGES], f16, tag="n0T")
            nc.vector.tensor_copy(n0T[:], n0f[:])

            # --- hop-2 segments -> n1T (rotating k-planes, chained accum) ---
            i2t = i2pool.tile([P, NSEG * F1 * CALL // 16], i16, tag="i2t")
            nc.sync.dma_start(i2t[:], (idx_s if sd == "s" else idx_d)[:])
            n1T = hpool.tile([P, 2, EDGES * F0], f16, tag="n1T")
            for s in range(NSEG):
                tabAB = tabs[sd + _SEG_TO_TAB[s]]
                acc = acpool.tile([P, SEG // P, D], f16, tag="acc")
                planes = []
                for k in range(F1):
                    pl = plpool.tile([P, SEG // P, D], f16, tag="pl")
                    co = (s * F1 + k) * (CALL // 16)
                    nc.gpsimd.dma_gather(
                        pl[:], tabAB[:, :], i2t[:, co:co + CALL // 16],
                        CALL, CALL, D, queue_num=qn[0] % 4)
                    qn[0] += 1
                    planes.append(pl)
                    if k == 1:
                        nc.vector.tensor_add(acc[:], planes[0][:], planes[1][:])
                    elif k > 1:
                        nc.vector.tensor_add(acc[:], acc[:], pl[:])
                n1s = acc
                # transpose to feat-major: 16 tiles packed 4-per-psum
                for f in range(2):
                    for g in range(2):
                        pt = pst.tile([P, 4, P], f16, tag="pt")
                        for b in range(4):
                            nc.tensor.matmul(
                                out=pt[:, b, :],
                                lhsT=n1s[:, g * 4 + b, f * P:(f + 1) * P],
                                rhs=ident[:], is_transpose=True)
                        dst = n1T[:, f, s * SEG + g * 512:
                                  s * SEG + (g + 1) * 512]
                        if (f + g) % 2 == 0:
                            nc.scalar.activation(out=dst, in_=pt[:].rearrange(
                                "p b q -> p (b q)"), func=AF.Copy)
                        else:
                            nc.vector.tensor_copy(dst, pt[:].rearrange(
                                "p b q -> p (b q)"))

            # --- g1 = relu(h1 @ Ws0 + n1 @ Wa0) ---
            g1T = gpool.tile([P, 2, EDGES * F0], f16, tag="g1T")
            for o in range(2):
                for bb in range(EDGES * F0 // 512):
                    sl = slice(bb * 512, (bb + 1) * 512)
                    ps = ps2.tile([P, 512], f32, tag="mmps")
                    for cch in range(2):
                        nc.tensor.matmul(
                            out=ps[:], lhsT=ws0[:, cch, o * P:(o + 1) * P],
                            rhs=h1T[:, cch, sl], start=(cch == 0), stop=False)
                        nc.tensor.matmul(
                            out=ps[:], lhsT=wa0[:, cch, o * P:(o + 1) * P],
                            rhs=n1T[:, cch, sl], start=False, stop=(cch == 1))
                    nc.scalar.activation(out=g1T[:, o, sl], in_=ps[:],
                                         func=AF.Relu)

            # --- nT = group-sums of g1T ---
            nf = vpool.tile([P, 2, EDGES], f32, tag="tr")
            tree10(g1T[:], nf[:], EDGES)
            nT = hpool.tile([P, 2, EDGES], f16, tag="nT")
            nc.vector.tensor_copy(nT[:], nf[:])

            # --- g0 = relu(h0 @ Ws0 + n0 @ Wa0) ---
            g0T = gpool.tile([P, 2, EDGES], f16, tag="g0T")
            for o in range(2):
                ps = ps2.tile([P, 512], f32, tag="mmps")
                for cch in range(2):
                    nc.tensor.matmul(out=ps[:],
                                     lhsT=ws0[:, cch, o * P:(o + 1) * P],
                                     rhs=h0T[:, cch, :], start=(cch == 0),
                                     stop=False)
                    nc.tensor.matmul(out=ps[:],
                                     lhsT=wa0[:, cch, o * P:(o + 1) * P],
                                     rhs=n0T[:, cch, :], start=False,
                                     stop=(cch == 1))
                nc.scalar.activation(out=g0T[:, o, :], in_=ps[:], func=AF.Relu)

            # --- hT = g0 @ Ws1 + n @ Wa1 ---
            ps = ps2.tile([P, 512], f32, tag="mmps")
            for cch in range(2):
                nc.tensor.matmul(out=ps[:], lhsT=ws1[:, cch, :],
                                 rhs=g0T[:, cch, :], start=(cch == 0),
                                 stop=False)
                nc.tensor.matmul(out=ps[:], lhsT=wa1[:, cch, :],
                                 rhs=nT[:, cch, :], start=False,
                                 stop=(cch == 1))
            hT = gpool.tile([P, EDGES], f16, tag=f"hT{sd}")
            nc.scalar.activation(out=hT[:], in_=ps[:], func=AF.Copy)
            hts[sd] = hT

        prod = gpool.tile([P, EDGES], f16, tag="prod")
        nc.vector.tensor_mul(prod[:], hts["s"][:], hts["d"][:])
        psf = ps2.tile([1, EDGES], f32, tag="mmps")
        nc.tensor.matmul(out=psf[:], lhsT=wlin_s[:], rhs=prod[:],
                         start=True, stop=True)
        res = gpool.tile([1, EDGES], f32, tag="res")
        nc.scalar.activation(out=res[:], in_=psf[:], func=AF.Identity,
                             bias=blin_s[:, :1])
        nc.sync.dma_start(out[:], res[:])

    nc.compile()
    return nc


def kernel(**inputs) -> np.ndarray:
    from concourse.bass_utils import run_bass_kernel_spmd

    plan = _build_plans(inputs)

    trace = bool(os.environ.get("GNN_TRACE"))
    if trace:
        import timing_shim
        timing_shim.install()

    nc = _build_bass(plan)

    f16 = np.float16
    # feature tables, feature-major fp16 [512, N]
    feat_u16 = np.ascontiguousarray(
        np.asarray(inputs["user_feat"], np.float32).T).astype(f16)
    feat_i16 = np.ascontiguousarray(
        np.asarray(inputs["item_feat"], np.float32).T).astype(f16)
    xts = _proj_host(feat_u16, feat_i16, plan)

    w_pu = np.ascontiguousarray(
        np.asarray(inputs["W_pu"], np.float32).reshape(4, P, D)
        .transpose(1, 0, 2)).astype(f16)
    w_pi = np.ascontiguousarray(
        np.asarray(inputs["W_pi"], np.float32).reshape(4, P, D)
        .transpose(1, 0, 2)).astype(f16)
    b_p = np.stack([np.tile(np.asarray(inputs["b_pu"], np.float32), 2),
                    np.tile(np.asarray(inputs["b_pi"], np.float32), 2)])[None].astype(f16)

    def sagew(pre):
        s0 = np.asarray(inputs[f"{pre}_self0"], np.float32)
        a0 = np.asarray(inputs[f"{pre}_agg0"], np.float32) * (1.0 / F0)
        s1 = np.asarray(inputs[f"{pre}_self1"], np.float32)
        a1 = np.asarray(inputs[f"{pre}_agg1"], np.float32) * (1.0 / F0)
        cat = np.concatenate([s0, a0, s1, a1], axis=1)  # [256, 768]
        return cat.reshape(2, P, 768).transpose(1, 0, 2)

    wsage = np.ascontiguousarray(
        np.concatenate([sagew("u"), sagew("i")], axis=2)).astype(f16)
    wlin = np.asarray(inputs["W_lin"], np.float32).astype(f16)
    blin = np.asarray(inputs["b_lin"], np.float32).reshape(1, 1)

    in_maps = []
    for c in range(NCORES):
        m = {
            "xt": xts[c], "w_pu": w_pu, "w_pi": w_pi, "b_p": b_p,
            "wsage": wsage, "wlin": wlin, "blin": blin,
            "idx_s": _wrap16(plan["idx2"][c]["s"]),
            "idx_d": _wrap16(plan["idx2"][c]["d"]),
        }
        in_maps.append(m)

    kw = dict(trace=True, trace_cores=list(range(NCORES))) if trace else {}
    res = run_bass_kernel_spmd(nc, in_maps, core_ids=list(range(NCORES)), **kw)
    if trace and res.exec_time_ns:
        print(f"HW exec time: {res.exec_time_ns} ns")
        kernel.last_exec_ns = res.exec_time_ns

    logits = np.concatenate([res.results[c]["out"][0] for c in range(NCORES)])
    return logits.reshape(B, 1).astype(np.float32)
